# revision 1
# baseline (speedup 1.0000x reference)
"""ChebConv (K=4) Trainium2 kernel: 8-core row-sharded SpMM + dense contraction.

Dataflow per core (rows [c*6250, (c+1)*6250) padded to 6272):
  x0 table (50176, 256) f32 in DRAM (host-built, padded global ids).
  3 Chebyshev SpMM steps: per 128-row tile, dma_gather the edge columns'
  z-rows from the table (int16 idx, lo/hi split at 32768), then reduce into
  the tile's 128 rows with per-128-slot indicator matmuls on the PE
  (lhsT M[slot, row] = val*(row_local[slot]==row), fp32r), accumulating in
  PSUM. Recurrence x_k = (2L)x_{k-1} - x_{k-2} via DVE subtract against the
  x_{k-2} tile re-read from its DRAM bounce. Per-step slices are cast to the
  gather dtype (bf16 by default) and AllGather'd into the next step's full
  table; fp32 slices stay in per-core DRAM bounces. Contraction phase:
  PE-transpose x_k tiles (fp32), then out[b].T (Cout, v) =
  sum_k W_k.T @ x_k[b].T with stationary fp32r weights, bias via tensor_scalar.
"""

import sys

sys.path.insert(0, "/opt/trn_rl_repo")

import numpy as np

V = 50000
E = 800000
B, CIN, COUT, K = 4, 64, 128, 4
NC = 8
VC = V // NC              # 6250
VCP = 6272                # 49*128 padded rows per core
TILES = VCP // 128        # 49
VPT = NC * VCP            # 50176 table rows
F = B * CIN               # 256
HI_BASE = 32768           # int16 index split


# ---------------- host-side preprocessing ----------------

def _remap_col(g):
    return (g // VC) * VCP + (g % VC)


def preprocess(rows, cols, vals):
    """Split edges per core/tile into lo/hi index halves, pad to uniform
    counts, emit gather indices + per-chunk indicator metadata."""
    rows = np.asarray(rows)
    cols_r = _remap_col(np.asarray(cols).astype(np.int64))
    vals = np.asarray(vals, dtype=np.float32)

    per_core = []
    max_lo = max_hi = 0
    for c in range(NC):
        lo_r, hi_r = c * VC, (c + 1) * VC
        m = (rows >= lo_r) & (rows < hi_r)
        r = rows[m] - lo_r
        cg = cols_r[m]
        vv = vals[m]
        t_of = r // 128
        tiles = []
        for t in range(TILES):
            sel = t_of == t
            rt, ct, vt = r[sel], cg[sel], vv[sel]
            isl = ct < HI_BASE
            tl = (rt[isl] - t * 128, ct[isl], vt[isl])
            th = (rt[~isl] - t * 128, ct[~isl] - HI_BASE, vt[~isl])
            max_lo = max(max_lo, len(tl[0]))
            max_hi = max(max_hi, len(th[0]))
            tiles.append((tl, th))
        per_core.append(tiles)

    NLO = -(-max_lo // 128) * 128
    NHI = -(-max_hi // 128) * 128
    CPT = (NLO + NHI) // 128          # chunks per tile
    NCH = TILES * CPT
    SPT16 = (NLO + NHI) // 16         # idx columns per tile

    cores = []
    for c in range(NC):
        idx = np.zeros((128, TILES * SPT16), dtype=np.int16)
        rl = np.zeros((128, NCH), dtype=np.float32)
        v1 = np.zeros((128, NCH), dtype=np.float32)
        for t in range(TILES):
            (rlo, clo, vlo), (rhi, chi, vhi) = per_core[c][t]
            # slot order within tile: lo edges, lo pads, hi edges, hi pads
            rr = np.zeros(NLO + NHI, np.float32)
            cc = np.zeros(NLO + NHI, np.int32)
            vv = np.zeros(NLO + NHI, np.float32)
            n = len(rlo)
            rr[:n], cc[:n], vv[:n] = rlo, clo, vlo
            nh = len(rhi)
            rr[NLO : NLO + nh], cc[NLO : NLO + nh], vv[NLO : NLO + nh] = rhi, chi, vhi
            # gather idx, 16-wrapped, replicated across the 8 Q7 groups
            w = cc.reshape(SPT16, 16).T.astype(np.int16)       # (16, SPT16)
            idx[:, t * SPT16 : (t + 1) * SPT16] = np.tile(w, (8, 1))
            # chunk metadata: slot i -> chunk i//128, partition i%128
            ch0 = t * CPT
            rl[:, ch0 : ch0 + CPT] = rr.reshape(CPT, 128).T
            v1[:, ch0 : ch0 + CPT] = vv.reshape(CPT, 128).T
        cores.append(dict(idx=idx, rl=rl, v1=v1, v2=2.0 * v1))
    return cores, NLO, NHI


def host_inputs(x, lap_rows, lap_cols, lap_vals, weight, bias):
    x0 = np.ascontiguousarray(np.transpose(x, (2, 0, 1)).reshape(V, F)).astype(np.float32)
    table0 = np.zeros((VPT, F), dtype=np.float32)
    for c in range(NC):
        table0[c * VCP : c * VCP + VC] = x0[c * VC : (c + 1) * VC]
    cores, NLO, NHI = preprocess(lap_rows, lap_cols, lap_vals)
    iota = np.broadcast_to(np.arange(128, dtype=np.float32)[None, :], (128, 128)).copy()
    wlo = np.zeros((128, K * COUT), np.float32)
    whi = np.zeros((128, K * COUT), np.float32)
    for k in range(K):
        wlo[0:64, k * COUT : (k + 1) * COUT] = weight[k]
        whi[64:128, k * COUT : (k + 1) * COUT] = weight[k]
    bias_t = np.asarray(bias, np.float32).reshape(128, 1)
    in_maps = []
    for c in range(NC):
        in_maps.append(
            dict(
                x0slice=np.ascontiguousarray(table0[c * VCP : (c + 1) * VCP]),
                idx=cores[c]["idx"],
                rl=cores[c]["rl"],
                v1=cores[c]["v1"],
                v2=cores[c]["v2"],
                iota=iota,
                wlo=wlo,
                whi=whi,
                bias=bias_t,
            )
        )
    return in_maps, NLO, NHI


# ---------------- device module ----------------

_CACHE = {}


def build_module(NLO, NHI, sim=False, ablate='full', bf16=True, NQ=4, deep=False):
    key = (NLO, NHI, sim, ablate, bf16, NQ, deep)
    if key in _CACHE:
        return _CACHE[key]
    from concourse import bass, mybir, bacc
    import concourse.tile as tile
    from concourse.masks import make_identity

    CPT = (NLO + NHI) // 128
    NCH = TILES * CPT
    SPT16 = (NLO + NHI) // 16
    BLO = NLO // 128
    f32, f32r, i16 = mybir.dt.float32, mybir.dt.float32r, mybir.dt.int16
    gdt = mybir.dt.bfloat16 if bf16 else f32r

    nc = bacc.Bacc("TRN2", target_bir_lowering=False, debug=False, num_devices=1 if sim else NC, num_swdge_queues=4)

    x0slice = nc.dram_tensor("x0slice", [VCP, F], f32, kind="ExternalInput")
    idx_in = nc.dram_tensor("idx", [128, TILES * SPT16], i16, kind="ExternalInput")
    rl_in = nc.dram_tensor("rl", [128, NCH], f32, kind="ExternalInput")
    v1_in = nc.dram_tensor("v1", [128, NCH], f32, kind="ExternalInput")
    v2_in = nc.dram_tensor("v2", [128, NCH], f32, kind="ExternalInput")
    iota_in = nc.dram_tensor("iota", [128, 128], f32, kind="ExternalInput")
    wlo_in = nc.dram_tensor("wlo", [128, K * COUT], f32, kind="ExternalInput")
    whi_in = nc.dram_tensor("whi", [128, K * COUT], f32, kind="ExternalInput")
    bias_in = nc.dram_tensor("bias", [128, 1], f32, kind="ExternalInput")
    out_t = nc.dram_tensor("out", [B, COUT, VCP], f32, kind="ExternalOutput")

    with tile.TileContext(nc) as tc:
        with (
            tc.tile_pool(name="pers", bufs=1) as pers,
            tc.tile_pool(name="gpool", bufs=3 if deep else 2) as gpool,
            tc.tile_pool(name="mval", bufs=8 if deep else 4) as mvpool,
            tc.tile_pool(name="spmm_ps", bufs=3 if deep else 2, space="PSUM") as pspool,
            tc.tile_pool(name="tp_ps", bufs=2, space="PSUM") as tppool,
            tc.tile_pool(name="out_ps", bufs=2, space="PSUM") as popool,
            tc.tile_pool(name="stage", bufs=3) as spool,
            tc.tile_pool(name="xt", bufs=5) as xtpool,
            tc.tile_pool(name="obuf", bufs=3) as obpool,
            tc.tile_pool(name="dram", bufs=1, space="DRAM") as dram,
        ):
            # persistent loads
            idx_t = pers.tile([128, TILES * SPT16], i16)
            nc.sync.dma_start(idx_t[:], idx_in[:])
            rl_t = pers.tile([128, NCH], f32)
            nc.sync.dma_start(rl_t[:], rl_in[:])
            v1_t = pers.tile([128, NCH], f32)
            nc.sync.dma_start(v1_t[:], v1_in[:])
            v2_t = pers.tile([128, NCH], f32)
            nc.sync.dma_start(v2_t[:], v2_in[:])
            iota_t = pers.tile([128, 128], f32)
            nc.sync.dma_start(iota_t[:], iota_in[:])
            wlo_t = pers.tile([128, K * COUT], f32r)
            nc.gpsimd.dma_start(wlo_t[:], wlo_in[:])
            whi_t = pers.tile([128, K * COUT], f32r)
            nc.gpsimd.dma_start(whi_t[:], whi_in[:])
            bias_t = pers.tile([128, 1], f32)
            nc.sync.dma_start(bias_t[:], bias_in[:])
            ident = pers.tile([128, 128], f32)
            make_identity(nc, ident[:])

            bounce = [dram.tile([VCP, F], f32, name=f"bounce{i}", tag=f"bounce{i}") for i in range(3)]
            tables = [dram.tile([VPT, F], gdt, name=f"table{i+1}", tag=f"table{i+1}") for i in range(2)]
            bncg = [dram.tile([VCP, F], gdt, name=f"bncg{i}", tag=f"bncg{i}") for i in range(2)]
            x0b = dram.tile([VCP, F], gdt, name="x0b", tag="x0b")
            table0 = dram.tile([VPT, F], gdt, name="table0i", tag="table0i")
            nc.gpsimd.dma_start(x0b[:], x0slice[:])  # cast f32 -> gather dtype
            if sim:
                nc.sync.dma_start(table0[0:VCP, :], x0b[:])
            else:
                nc.gpsimd.collective_compute(
                    "AllGather", mybir.AluOpType.bypass,
                    replica_groups=[list(range(NC))],
                    ins=[x0b.opt()], outs=[table0.opt()],
                )

            # ---------- contraction (emitted interleaved with step 3) ----------
            srcs = [x0slice, bounce[0], bounce[1], bounce[2]]
            vblocks = [(i * 512, 512) for i in range(VCP // 512)]
            if VCP % 512:
                vblocks.append((VCP // 512 * 512, VCP % 512))
            if ablate == 'nophase2':
                vblocks = []

            def phase2_vblock(v0, nv):
                nq = nv // 128
                xts = []
                for k in range(K):
                    stage = spool.tile([128, 4 * F], f32, tag="stage")
                    nc.sync.dma_start(
                        stage[:, : nq * F].rearrange("p (q f) -> p q f", f=F),
                        srcs[k][v0 : v0 + nv, :].rearrange("(q p) f -> p q f", p=128),
                    )
                    xt_lo = xtpool.tile([128, 512], f32r, tag="xtlo")
                    xt_hi = xtpool.tile([128, 512], f32r, tag="xthi")
                    for q in range(nq):
                        for h in range(2):
                            tp = tppool.tile([128, 128], mybir.dt.float32, space="PSUM")
                            nc.tensor.transpose(
                                out=tp[:], in_=stage[:, q * F + h * 128 : q * F + (h + 1) * 128],
                                identity=ident[:],
                            )
                            dst = xt_lo if h == 0 else xt_hi
                            nc.any.tensor_copy(out=dst[:, q * 128 : (q + 1) * 128], in_=tp[:])
                    xts.append((xt_lo, xt_hi))
                for b in range(B):
                    h, off = divmod(b, 2)
                    off *= 64
                    wt = wlo_t if off == 0 else whi_t
                    po = popool.tile([128, 512], mybir.dt.float32, space="PSUM")
                    for k in range(K):
                        xt = xts[k][h]
                        nc.tensor.matmul(
                            out=po[:, :nv], lhsT=wt[off : off + 64, k * COUT : (k + 1) * COUT],
                            rhs=xt[off : off + 64, :nv], start=(k == 0), stop=(k == K - 1),
                        )
                    ob = obpool.tile([128, 512], f32, tag="ob")
                    nc.any.tensor_scalar_add(ob[:, :nv], po[:, :nv], bias_t[:, 0:1])
                    nc.sync.dma_start(out_t[b, :, v0 : v0 + nv], ob[:, :nv])


            # ---------- SpMM steps ----------
            for k in (1, 2, 3):
                src = table0 if k == 1 else tables[k - 2]
                vmeta = v1_t if k == 1 else v2_t
                prev_src = None if k == 1 else (x0slice if k == 2 else bounce[0])
                for t in range(TILES):
                    gt = gpool.tile([128, CPT * F], gdt, tag="G")
                    c0 = t * SPT16
                    # Q7 scratch caps one gather at ~1024 idxs; split into <=896 segs
                    segs = []
                    for base, n, hi in ((0, NLO, False), (BLO * 128, NHI, True)):
                        done = 0
                        while done < n:
                            m = min(896, n - done)
                            segs.append((base + done, m, hi))
                            done += m
                    for si, (off, n, hi) in enumerate(segs):
                        if ablate == 'nogather':
                            continue
                        sap = src[HI_BASE:, :] if hi else src[:]
                        nc.gpsimd.dma_gather(
                            out_ap=gt[:, off * 2 : (off + n) * 2].rearrange(
                                "p (j f) -> p j f", f=F),
                            in_ap=sap if bf16 else sap.bitcast(f32r),
                            idxs_ap=idx_t[:, c0 + off // 16 : c0 + (off + n) // 16],
                            num_idxs=n, num_idxs_reg=n, elem_size=F,
                            single_packet=False, queue_num=(t * len(segs) + si) % NQ,
                        )
                    ps = pspool.tile([128, F], mybir.dt.float32, space="PSUM")
                    for j in range(CPT if ablate not in ('nomm',) else 1):
                        ch = t * CPT + j
                        mv = mvpool.tile([128, 128], gdt)
                        nc.any.tensor_scalar(
                            out=mv[:], in0=iota_t[:],
                            scalar1=rl_t[:, ch : ch + 1], scalar2=vmeta[:, ch : ch + 1],
                            op0=mybir.AluOpType.is_equal, op1=mybir.AluOpType.mult,
                        )
                        nc.tensor.matmul(
                            out=ps[:], lhsT=mv[:], rhs=gt[:, j * F : (j + 1) * F],
                            start=(j == 0), stop=(j == CPT - 1),
                        )
                    xo = spool.tile([128, F], f32, tag="xout")
                    if k == 1:
                        nc.any.tensor_copy(out=xo[:], in_=ps[:])
                    else:
                        xp = spool.tile([128, F], f32, tag="xprev")
                        nc.sync.dma_start(xp[:], prev_src[t * 128 : (t + 1) * 128, :])
                        nc.vector.tensor_tensor(out=xo[:], in0=ps[:], in1=xp[:], op=mybir.AluOpType.subtract)
                    nc.sync.dma_start(bounce[k - 1][t * 128 : (t + 1) * 128, :], xo[:])
                    if k == 3 and t % 4 == 3 and ablate != 'nophase2':
                        phase2_vblock(t // 4 * 512, 512)
                    if bf16 and k < 3:
                        xg = spool.tile([128, F], gdt, tag="xg")
                        nc.any.tensor_copy(out=xg[:], in_=xo[:])
                        nc.sync.dma_start(bncg[k - 1][t * 128 : (t + 1) * 128, :], xg[:])
                if k < 3:
                    agin = bncg[k - 1] if bf16 else bounce[k - 1].bitcast(f32r)
                    if sim:
                        nc.sync.dma_start(tables[k - 1][0:VCP, :], agin[:])
                    else:
                        nc.gpsimd.collective_compute(
                            "AllGather", mybir.AluOpType.bypass,
                            replica_groups=[list(range(NC))],
                            ins=[agin.opt()], outs=[tables[k - 1].opt()],
                        )

            # ragged tail vblock(s) not covered by the interleaved emission
            for v0, nv in vblocks:
                if nv != 512:
                    phase2_vblock(v0, nv)

    nc.compile()
    _CACHE[key] = nc
    return nc


# ---------------- entry point ----------------

def kernel(x, lap_rows, lap_cols, lap_vals, weight, bias):
    from concourse.bass_utils import run_bass_kernel_spmd

    x = np.asarray(x, np.float32)
    weight = np.asarray(weight, np.float32)
    bias = np.asarray(bias, np.float32)
    in_maps, NLO, NHI = host_inputs(x, lap_rows, lap_cols, lap_vals, weight, bias)
    nc = build_module(NLO, NHI)
    res = run_bass_kernel_spmd(nc, in_maps, core_ids=list(range(NC)))
    out = np.empty((B, COUT, V), np.float32)
    for c in range(NC):
        out[:, :, c * VC : (c + 1) * VC] = res.results[c]["out"][:, :, :VC]
    return out



# revision 20
# speedup vs baseline: 1.9703x; 1.9703x over previous
"""ChebConv (K=4) Trainium2 kernel: 8-core row-sharded SpMM + dense contraction.

v2 dataflow, per core (rows load-balanced into 49 tiles of 128):
  Host: rows are permuted per core so every 128-row tile carries ~equal lo/hi
  edge counts (LPT balance), then each tile's edges are packed full-row into
  128-slot chunks (lo stream = cols < 32768 in the padded table, hi stream =
  rest; int16 gather indices address the hi stream via a +32768 base view).
  The initial x0 table (50176, 256) bf16 is host-built and staged as input.

  Device, 3 Chebyshev SpMM steps: per tile, 2 dma_gathers (lo/hi) fetch the
  edge columns' rows from the bf16 table; per 128-slot chunk an indicator
  matrix M[slot, row] = val*(row_local[slot]==row) (bf16, built by
  tensor_scalar from iota/rl/val metadata) reduces into the tile's 128 rows
  on the PE, accumulating in PSUM. Recurrence x_k = (2L)x_{k-1} - x_{k-2} via
  DVE subtract against the bf16 slice of x_{k-2} re-read from its DRAM
  bounce. Per-step bf16 slices are AllGather'd (Shared-scratchpad output =
  local HBM writes + rendezvous, no D2D hop) into the next step's table.
  Contraction: PE-transpose x_k tiles (bf16), out[b].T = sum_k W_k.T @
  x_k[b].T with bf16 weights, bias via tensor_scalar, interleaved with the
  last SpMM step.
"""

import sys

sys.path.insert(0, "/opt/trn_rl_repo")

import numpy as np

V = 50000
E = 800000
B, CIN, COUT, K = 4, 64, 128, 4
NC = 8
VC = V // NC              # 6250
VCP = 6272                # 49*128 padded rows per core
TILES = VCP // 128        # 49
VPT = NC * VCP            # 50176 table rows
F = B * CIN               # 256
HI_BASE = 32768           # int16 index split


# ---------------- host-side preprocessing ----------------

def _balance_rows(deg_lo, deg_hi):
    """LPT-pack 6250 rows (+22 pads) into 49 bins of exactly 128 rows,
    balancing lo- and hi-stream edge counts jointly.
    Returns perm[t*128+i] = local row id (or -1)."""
    deg_tot = deg_lo + deg_hi
    slo = max(float(deg_lo.sum()) / TILES, 1.0)
    shi = max(float(deg_hi.sum()) / TILES, 1.0)
    order = np.argsort(-deg_tot, kind="stable")
    lo_loads = np.zeros(TILES)
    hi_loads = np.zeros(TILES)
    counts = np.zeros(TILES, dtype=np.int64)
    bins = [[] for _ in range(TILES)]
    big = np.float64(1e18)
    for r in order:
        score = np.maximum((lo_loads + deg_lo[r]) / slo, (hi_loads + deg_hi[r]) / shi)
        score = np.where(counts < 128, score, big)
        t = int(np.argmin(score))
        bins[t].append(int(r))
        lo_loads[t] += deg_lo[r]
        hi_loads[t] += deg_hi[r]
        counts[t] += 1
    perm = np.full(VCP, -1, dtype=np.int64)
    for t in range(TILES):
        for i, r in enumerate(bins[t]):
            perm[t * 128 + i] = r
    return perm


def _pack_rows(rows_in_tile, by_row):
    """FFD full-row packing of (row_pos -> edge list) into 128-slot chunks.
    Returns list of chunks; each chunk = list of (row_pos, col, val)."""
    items = []
    for rp in rows_in_tile:
        ent = by_row.get(rp)
        if ent:
            items.append((len(ent), rp, ent))
    items.sort(key=lambda x: -x[0])
    chunks, space = [], []
    for deg, rp, ent in items:
        placed = False
        for ci in range(len(chunks)):
            if space[ci] >= deg:
                chunks[ci].extend((rp, c, v) for c, v in ent)
                space[ci] -= deg
                placed = True
                break
        if not placed:
            chunks.append([(rp, c, v) for c, v in ent])
            space.append(128 - deg)
    return chunks


def preprocess(rows, cols, vals):
    """Balance + staircase-pack edges; emit per-core gather idx streams and
    per-chunk indicator metadata + the static per-tile chunk structure."""
    rows = np.asarray(rows)
    cols = np.asarray(cols).astype(np.int64)
    vals = np.asarray(vals, dtype=np.float32)

    # row balance per core (degree over both streams)
    perms, pos_of = [], []
    core_edges = []
    for c in range(NC):
        lo_r, hi_r = c * VC, (c + 1) * VC
        m = (rows >= lo_r) & (rows < hi_r)
        r = (rows[m] - lo_r).astype(np.int64)
        core_edges.append((r, cols[m], vals[m]))
        # lo/hi ~ source core <= 4 (core 5's sliver misclassified; balance-only)
        src_lo = (cols[m] // VC) <= 4
        deg_lo = np.bincount(r[src_lo], minlength=VC)
        deg_hi = np.bincount(r[~src_lo], minlength=VC)
        perm = _balance_rows(deg_lo, deg_hi)
        perms.append(perm)
        inv = np.full(VC, -1, dtype=np.int64)
        val_positions = np.nonzero(perm >= 0)[0]
        inv[perm[val_positions]] = val_positions
        pos_of.append(inv)
    pos_all = np.concatenate(pos_of)                     # (V,) position within core
    col_table = (cols // VC) * VCP + pos_all[cols]       # global table row per edge

    # per (core, tile): split lo/hi by table row, pack full rows into chunks
    packed = [[None] * TILES for _ in range(NC)]
    NLOC = np.zeros(TILES, dtype=np.int64)
    NHIC = np.zeros(TILES, dtype=np.int64)
    for c in range(NC):
        r, _, vv = core_edges[c]
        ct = col_table[(rows >= c * VC) & (rows < (c + 1) * VC)]
        rpos = pos_of[c][r]                              # position 0..6271
        t_of = rpos // 128
        rr = rpos % 128
        islo = ct < HI_BASE
        for t in range(TILES):
            sel = t_of == t
            chunks_2 = []
            for sl, base in ((islo & sel, 0), ((~islo) & sel, HI_BASE)):
                by_row = {}
                for rp, cc, vx in zip(rr[sl], ct[sl] - base, vv[sl]):
                    by_row.setdefault(int(rp), []).append((int(cc), float(vx)))
                chunks_2.append(_pack_rows(range(128), by_row))
            packed[c][t] = chunks_2
            NLOC[t] = max(NLOC[t], len(chunks_2[0]))
            NHIC[t] = max(NHIC[t], len(chunks_2[1]))

    CPT = NLOC + NHIC
    CHOFF = np.concatenate([[0], np.cumsum(CPT)])
    NCH = int(CHOFF[-1])

    cores = []
    for c in range(NC):
        idx = np.zeros((128, 8 * NCH), dtype=np.int16)
        rl = np.zeros((128, NCH), dtype=np.float32)
        v1 = np.zeros((128, NCH), dtype=np.float32)
        gcnt = np.zeros((1, 2 * TILES), dtype=np.int32)
        for t in range(TILES):
            lo_chunks, hi_chunks = packed[c][t]
            assert lo_chunks and hi_chunks, f"empty stream core {c} tile {t}"
            gcnt[0, 2 * t] = 128 * (len(lo_chunks) - 1) + len(lo_chunks[-1])
            gcnt[0, 2 * t + 1] = 128 * (len(hi_chunks) - 1) + len(hi_chunks[-1])
            for kind, chunk_list, off in ((0, lo_chunks, 0), (1, hi_chunks, NLOC[t])):
                for li, chunk in enumerate(chunk_list):
                    ch = int(CHOFF[t] + off + li)
                    cc = np.zeros(128, np.int32)
                    rrv = np.zeros(128, np.float32)
                    vvv = np.zeros(128, np.float32)
                    n = len(chunk)
                    assert n <= 128
                    cc[:n] = [e[1] for e in chunk]
                    rrv[:n] = [e[0] for e in chunk]
                    vvv[:n] = [e[2] for e in chunk]
                    w = cc.reshape(8, 16).T.astype(np.int16)      # (16, 8)
                    idx[:, 8 * ch : 8 * ch + 8] = np.tile(w, (8, 1))
                    rl[:, ch] = rrv
                    v1[:, ch] = vvv
        cores.append(dict(idx=idx, rl=rl, v1=v1, v2=2.0 * v1, gcnt=gcnt))
    meta = (tuple(int(x) for x in NLOC), tuple(int(x) for x in NHIC))
    return cores, perms, meta


def host_inputs(x, lap_rows, lap_cols, lap_vals, weight, bias):
    import ml_dtypes

    x0 = np.ascontiguousarray(np.transpose(x, (2, 0, 1)).reshape(V, F)).astype(np.float32)
    cores, perms, meta = preprocess(lap_rows, lap_cols, lap_vals)
    bf16 = ml_dtypes.bfloat16
    table0 = np.zeros((VPT, F), dtype=bf16)
    x0gs = []
    for c in range(NC):
        sl = np.zeros((VCP, F), dtype=bf16)
        valid = perms[c] >= 0
        sl[valid] = x0[c * VC + perms[c][valid]].astype(bf16)
        x0gs.append(sl)
        table0[c * VCP : (c + 1) * VCP] = sl
    iota = np.broadcast_to(np.arange(128, dtype=np.float32)[None, :], (128, 128)).copy()
    wlo = np.zeros((128, K * COUT), np.float32)
    whi = np.zeros((128, K * COUT), np.float32)
    for k in range(K):
        wlo[0:64, k * COUT : (k + 1) * COUT] = weight[k]
        whi[64:128, k * COUT : (k + 1) * COUT] = weight[k]
    bias_t = np.asarray(bias, np.float32).reshape(128, 1)
    in_maps = []
    for c in range(NC):
        in_maps.append(
            dict(
                x0g=x0gs[c],
                table0=table0,
                idx=cores[c]["idx"],
                rl=cores[c]["rl"],
                v1=cores[c]["v1"],
                v2=cores[c]["v2"],
                gcnt=cores[c]["gcnt"],
                iota=iota,
                wlo=wlo.astype(bf16),
                whi=whi.astype(bf16),
                bias=bias_t,
            )
        )
    return in_maps, perms, meta


# ---------------- device module ----------------

_CACHE = {}


def build_module(meta, sim=False, shared=True, NQ=4):
    key = (meta, sim, shared, NQ)
    if key in _CACHE:
        return _CACHE[key]
    from concourse import bass, mybir, bacc
    import concourse.tile as tile
    from concourse.masks import make_identity

    NLOC, NHIC = (np.asarray(m, dtype=np.int64) for m in meta)
    CPT = NLOC + NHIC
    CHOFF = np.concatenate([[0], np.cumsum(CPT)])
    NCH = int(CHOFF[-1])
    CPTMAX = int(CPT.max())
    f32, i16 = mybir.dt.float32, mybir.dt.int16
    bf16 = mybir.dt.bfloat16

    nc = bacc.Bacc("TRN2", target_bir_lowering=False, debug=False,
                   num_devices=1 if sim else NC, num_swdge_queues=NQ,
                   dynamic_dma_scratch_size=32768)

    x0g_in = nc.dram_tensor("x0g", [VCP, F], bf16, kind="ExternalInput")
    table0 = nc.dram_tensor("table0", [VPT, F], bf16, kind="ExternalInput")
    idx_in = nc.dram_tensor("idx", [128, 8 * NCH], i16, kind="ExternalInput")
    rl_in = nc.dram_tensor("rl", [128, NCH], f32, kind="ExternalInput")
    v1_in = nc.dram_tensor("v1", [128, NCH], f32, kind="ExternalInput")
    v2_in = nc.dram_tensor("v2", [128, NCH], f32, kind="ExternalInput")
    gcnt_in = nc.dram_tensor("gcnt", [1, 2 * TILES], mybir.dt.int32, kind="ExternalInput")
    iota_in = nc.dram_tensor("iota", [128, 128], f32, kind="ExternalInput")
    wlo_in = nc.dram_tensor("wlo", [128, K * COUT], bf16, kind="ExternalInput")
    whi_in = nc.dram_tensor("whi", [128, K * COUT], bf16, kind="ExternalInput")
    bias_in = nc.dram_tensor("bias", [128, 1], f32, kind="ExternalInput")
    out_t = nc.dram_tensor("out", [B, COUT, VCP], bf16, kind="ExternalOutput")

    with tile.TileContext(nc) as tc:
        with (
            tc.tile_pool(name="pers", bufs=1) as pers,
            tc.tile_pool(name="gpool", bufs=3) as gpool,
            tc.tile_pool(name="mval", bufs=8) as mvpool,
            tc.tile_pool(name="spmm_ps", bufs=3, space="PSUM") as pspool,
            tc.tile_pool(name="tp_ps", bufs=2, space="PSUM") as tppool,
            tc.tile_pool(name="out_ps", bufs=2, space="PSUM") as popool,
            tc.tile_pool(name="xt", bufs=5) as xtpool,
            tc.tile_pool(name="obuf", bufs=3) as obpool,
            tc.tile_pool(name="dram", bufs=1, space="DRAM") as dram,
        ):
            # persistent loads
            idx_t = pers.tile([128, 8 * NCH], i16)
            nc.sync.dma_start(idx_t[:], idx_in[:])
            rl_t = pers.tile([128, NCH], f32)
            nc.sync.dma_start(rl_t[:], rl_in[:])
            v1_t = pers.tile([128, NCH], f32)
            nc.sync.dma_start(v1_t[:], v1_in[:])
            v2_t = pers.tile([128, NCH], f32)
            nc.sync.dma_start(v2_t[:], v2_in[:])
            gcnt_t = pers.tile([1, 2 * TILES], mybir.dt.int32)
            nc.sync.dma_start(gcnt_t[:], gcnt_in[:])
            iota_t = pers.tile([128, 128], f32)
            nc.sync.dma_start(iota_t[:], iota_in[:])
            wlo_t = pers.tile([128, K * COUT], bf16)
            nc.sync.dma_start(wlo_t[:], wlo_in[:])
            whi_t = pers.tile([128, K * COUT], bf16)
            nc.sync.dma_start(whi_t[:], whi_in[:])
            bias_t = pers.tile([128, 1], f32)
            nc.sync.dma_start(bias_t[:], bias_in[:])
            ident = pers.tile([128, 128], bf16)
            make_identity(nc, ident[:])

            # SBUF-resident x_k slices, [128 rows, TILES*F]
            xn = [pers.tile([128, TILES * F], bf16, name=f"xn{k}") for k in range(K)]
            nc.sync.dma_start(
                xn[0][:].rearrange("p (t f) -> p t f", f=F),
                x0g_in[:].rearrange("(t p) f -> p t f", p=128),
            )

            aspace = "Shared" if shared else "Local"
            bncg = [dram.tile([VCP, F], bf16, name=f"bncg{i}", tag=f"bncg{i}") for i in range(2)]
            tables = [dram.tile([VPT, F], bf16, name=f"table{i+1}", tag=f"table{i+1}", addr_space=aspace) for i in range(2)]

            # ---------- contraction (emitted interleaved with step 3) ----------
            vblocks = [(i * 512, 512) for i in range(VCP // 512)]
            if VCP % 512:
                vblocks.append((VCP // 512 * 512, VCP % 512))

            def phase2_vblock(v0, nv):
                nq = nv // 128
                t0 = v0 // 128
                xts = []
                for k in range(K):
                    xt_lo = xtpool.tile([128, 512], bf16, tag="xtlo")
                    xt_hi = xtpool.tile([128, 512], bf16, tag="xthi")
                    for h in range(2):
                        tp = tppool.tile([128, 512], bf16, space="PSUM")
                        for q in range(nq):
                            nc.tensor.transpose(
                                out=tp[:, q * 128 : (q + 1) * 128],
                                in_=xn[k][:, (t0 + q) * F + h * 128 : (t0 + q) * F + (h + 1) * 128],
                                identity=ident[:],
                            )
                        dst = xt_lo if h == 0 else xt_hi
                        nc.any.tensor_copy(out=dst[:, : nq * 128], in_=tp[:, : nq * 128])
                    xts.append((xt_lo, xt_hi))
                for b in range(B):
                    h, off = divmod(b, 2)
                    off *= 64
                    wt = wlo_t if off == 0 else whi_t
                    po = popool.tile([128, 512], mybir.dt.float32, space="PSUM")
                    for k in range(K):
                        xt = xts[k][h]
                        nc.tensor.matmul(
                            out=po[:, :nv], lhsT=wt[off : off + 64, k * COUT : (k + 1) * COUT],
                            rhs=xt[off : off + 64, :nv], start=(k == 0), stop=(k == K - 1),
                        )
                    ob = obpool.tile([128, 512], bf16, tag="ob")
                    nc.any.tensor_scalar_add(ob[:, :nv], po[:, :nv], bias_t[:, 0:1])
                    nc.sync.dma_start(out_t[b, :, v0 : v0 + nv], ob[:, :nv])

            # ---------- SpMM steps ----------
            # stale (un-gathered) gt slots are killed by val=0 indicators, but
            # must be finite: zero the pool bufs once before first use
            for _ in range(3):
                gz = gpool.tile([128, CPTMAX * F], bf16, tag="G")
                nc.vector.memset(gz[:], 0)
            rcnt = nc.gpsimd.alloc_register("rcnt")
            qn = 0
            for k in (1, 2, 3):
                src = table0 if k == 1 else tables[k - 2]
                vmeta = v1_t if k == 1 else v2_t
                for t in range(TILES):
                    cpt_t = int(CPT[t])
                    ch0 = int(CHOFF[t])
                    gt = gpool.tile([128, CPTMAX * F], bf16, tag="G")
                    for si, (off_ch, n_ch, hi) in enumerate(
                            ((0, int(NLOC[t]), False), (int(NLOC[t]), int(NHIC[t]), True))):
                        n = n_ch * 128
                        if n == 0:
                            continue
                        sap = src[HI_BASE:, :] if hi else src[:]
                        nc.gpsimd.reg_load(rcnt, gcnt_t[0:1, 2 * t + si : 2 * t + si + 1])
                        nc.gpsimd.dma_gather(
                            out_ap=gt[:, off_ch * F : (off_ch + n_ch) * F].rearrange(
                                "p (j f) -> p j f", f=F),
                            in_ap=sap,
                            idxs_ap=idx_t[:, 8 * (ch0 + off_ch) : 8 * (ch0 + off_ch + n_ch)],
                            num_idxs=n, num_idxs_reg=rcnt, elem_size=F,
                            single_packet=False, queue_num=qn % NQ,
                        )
                        qn += 1
                    ps = pspool.tile([128, F], mybir.dt.float32, space="PSUM")
                    for j in range(cpt_t):
                        ch = ch0 + j
                        mv = mvpool.tile([128, 128], bf16)
                        nc.any.tensor_scalar(
                            out=mv[:], in0=iota_t[:],
                            scalar1=rl_t[:, ch : ch + 1], scalar2=vmeta[:, ch : ch + 1],
                            op0=mybir.AluOpType.is_equal, op1=mybir.AluOpType.mult,
                        )
                        nc.tensor.matmul(
                            out=ps[:], lhsT=mv[:], rhs=gt[:, j * F : (j + 1) * F],
                            start=(j == 0), stop=(j == cpt_t - 1),
                        )
                    xslot = xn[k][:, t * F : (t + 1) * F]
                    if k == 1:
                        nc.any.tensor_copy(out=xslot, in_=ps[:])
                    else:
                        nc.vector.tensor_tensor(
                            out=xslot, in0=ps[:], in1=xn[k - 2][:, t * F : (t + 1) * F],
                            op=mybir.AluOpType.subtract)
                    if k < 3:
                        nc.sync.dma_start(bncg[k - 1][t * 128 : (t + 1) * 128, :], xslot)
                    if k == 3 and t % 4 == 3:
                        phase2_vblock(t // 4 * 512, 512)
                if k < 3:
                    if sim:
                        nc.sync.dma_start(tables[k - 1][0:VCP, :], bncg[k - 1][:])
                    else:
                        nc.gpsimd.collective_compute(
                            "AllGather", mybir.AluOpType.bypass,
                            replica_groups=[list(range(NC))],
                            ins=[bncg[k - 1].opt()], outs=[tables[k - 1].opt()],
                        )

            # ragged tail vblock(s) not covered by the interleaved emission
            for v0, nv in vblocks:
                if nv != 512:
                    phase2_vblock(v0, nv)

    nc.compile()
    _CACHE[key] = nc
    return nc


# ---------------- entry point ----------------

def kernel(x, lap_rows, lap_cols, lap_vals, weight, bias):
    from concourse.bass_utils import run_bass_kernel_spmd

    x = np.asarray(x, np.float32)
    weight = np.asarray(weight, np.float32)
    bias = np.asarray(bias, np.float32)
    in_maps, perms, meta = host_inputs(x, lap_rows, lap_cols, lap_vals, weight, bias)
    nc = build_module(meta)
    res = run_bass_kernel_spmd(nc, in_maps, core_ids=list(range(NC)))
    out = np.empty((B, COUT, V), np.float32)
    for c in range(NC):
        valid = perms[c] >= 0
        out[:, :, c * VC + perms[c][valid]] = res.results[c]["out"][:, :, valid].astype(np.float32)
    return out
